# revision 7
# baseline (speedup 1.0000x reference)
"""Trainium2 Bass kernel for CINConv-style GNN message passing.

Strategy (8 NeuronCores, data parallel over destination nodes):
  - Core c owns nodes [c*6250, (c+1)*6250). Edges are partitioned by their
    destination shard on the host, then bucketed by (type, 128-node block).
  - Source rows are gathered from a bf16 copy of x with `dma_gather`
    (Q7 SWDGE gather, int16 indices -> x is addressed as two halves), 128
    edges per chunk, and aggregated into PSUM with one-hot matmuls:
        agg^T[d, n] += feat[e, d]^T @ onehot[e, n]
    so aggregates land directly in transposed [feat, node] layout.
  - Self terms (the (1+eps)*x contributions) ride along as self-edges.
  - The per-node MLPs run in transposed layout; second-layer weights are
    fused into the output projection on the host:
        out = relu(h1b @ (bW2 oW_b) + h1r @ (rW2 oW_r) + h1u @ (uW2 oW_u) + ob')
  - Results are PE-transposed back to [node, feat] and stored.
"""

import numpy as np
import ml_dtypes

import concourse.bass as bass
import concourse.mybir as mybir
from concourse import bacc
from concourse.tile import TileContext
from concourse.bass_utils import run_bass_kernel_spmd

bf16 = ml_dtypes.bfloat16
F32 = mybir.dt.float32
BF16 = mybir.dt.bfloat16
I16 = mybir.dt.int16

# ---- problem config (hardcoded) ----
N, E, D = 50000, 800000, 128
NC = 8
BLK = 128
PAD_DST = 200.0
SPLIT = 32768  # int16 index limit: x rows addressed as [0,SPLIT) + [SPLIT,N)
BOUNDARY, UPPER, REWIRE = 0, 1, 2
NT = 4  # chunk types: B, R, U1 (src half of upper msg), U2 (upper_ind half)

LAST_EXEC_NS = None
LAST_TRACE_PATH = None


def _cfg(n, n_cores):
    shard = n // n_cores
    nblk = -(-shard // BLK)
    return shard, nblk, nblk * BLK


# ---------------------------------------------------------------- host prep
def preprocess(src, dst, et, ui, n, n_cores, sb_blocks=4):
    """Bucket edges by (core, type, block); split by source half; build the
    shared chunk schedule plus per-core gather-index / one-hot-dst tensors."""
    shard, nblk, _ = _cfg(n, n_cores)
    core_of = dst // shard
    dloc = dst - core_of * shard
    blk = dloc // BLK
    doff = dloc - blk * BLK

    tmap = np.full(3, -1, np.int64)
    tmap[BOUNDARY], tmap[REWIRE], tmap[UPPER] = 0, 1, 2
    t_of = tmap[et]

    key = (core_of * 3 + t_of) * nblk + blk
    order = np.argsort(key, kind="stable")
    key_s = key[order]
    src_s, doff_s, ui_s = src[order], doff[order], ui[order]
    starts = np.searchsorted(key_s, np.arange(n_cores * 3 * nblk + 1))

    def bucket(c, t, b):
        i0, i1 = starts[(c * 3 + t) * nblk + b], starts[(c * 3 + t) * nblk + b + 1]
        return i0, i1

    # per (core, chunk-type, block, half) edge (val, dst) lists
    # chunk types: 0=B (self+boundary), 1=R (self+rewire), 2=U1 (src),
    #              3=U2 (upper_ind); halves: 0=lo (<SPLIT), 1=hi
    lists = {}
    cnt = np.zeros((n_cores, NT, nblk, 2), np.int64)
    for c in range(n_cores):
        base = c * shard
        for b in range(nblk):
            n0 = b * BLK
            nreal = max(0, min(BLK, shard - n0))
            self_src = base + n0 + np.arange(nreal)
            self_dof = np.arange(nreal)
            i0, i1 = bucket(c, 0, b)
            vB = np.concatenate([self_src, src_s[i0:i1]])
            dB = np.concatenate([self_dof, doff_s[i0:i1]])
            i0, i1 = bucket(c, 1, b)
            vR = np.concatenate([self_src, src_s[i0:i1]])
            dR = np.concatenate([self_dof, doff_s[i0:i1]])
            i0, i1 = bucket(c, 2, b)
            vU1, dU1 = src_s[i0:i1], doff_s[i0:i1]
            vU2, dU2 = ui_s[i0:i1], doff_s[i0:i1]
            for t, (v, dd) in enumerate([(vB, dB), (vR, dR), (vU1, dU1), (vU2, dU2)]):
                m = v < SPLIT
                lists[(c, t, b, 0)] = (v[m], dd[m])
                lists[(c, t, b, 1)] = (v[~m] - SPLIT, dd[~m])
                cnt[c, t, b, 0] = m.sum()
                cnt[c, t, b, 1] = (~m).sum()

    # shared schedule: chunks per (type, block, half), >=1 chunk per (t,b)
    k = -(-cnt.max(axis=0) // BLK)          # [NT, nblk, 2]
    empty = k.sum(axis=2) == 0
    k[:, :, 0][empty] = 1

    # column layout: per super-block: [lo cols (b-major, t-minor)][hi cols]
    sb_bounds = list(range(0, nblk, sb_blocks)) + [nblk]
    nsb_count = len(sb_bounds) - 1
    cols = {}            # (b, t) -> list of global slab/dst cols (lo then hi)
    sb_info = []         # per sb: (col0, Klo, Khi, gidx_lo0, gidx_hi0)
    col = 0
    gcol = 0
    for s in range(nsb_count):
        b0, b1 = sb_bounds[s], sb_bounds[s + 1]
        col0 = col
        for b in range(b0, b1):
            for t in range(NT):
                cols[(b, t)] = [col + j for j in range(int(k[t, b, 0]))]
                col += int(k[t, b, 0])
        Klo = col - col0
        for b in range(b0, b1):
            for t in range(NT):
                cols[(b, t)] = cols[(b, t)] + [col + j
                                               for j in range(int(k[t, b, 1]))]
                col += int(k[t, b, 1])
        Khi = col - col0 - Klo
        sb_info.append((col0, Klo, Khi, gcol, gcol + Klo * 8))
        gcol += (Klo + Khi) * 8
    K_TOT = col

    gidx = np.zeros((n_cores, BLK, K_TOT * 8), np.int16)
    dst_t = np.full((n_cores, BLK, K_TOT), PAD_DST, bf16)

    for c in range(n_cores):
        for s in range(nsb_count):
            b0, b1 = sb_bounds[s], sb_bounds[s + 1]
            col0, Klo, Khi, glo, ghi = sb_info[s]
            for half, Kh, g0, hoff in [(0, Klo, glo, 0), (1, Khi, ghi, Klo)]:
                if Kh == 0:
                    continue
                vals = np.zeros(Kh * BLK, np.int16)
                dsts = np.full(Kh * BLK, PAD_DST, np.float32)
                p = 0
                for b in range(b0, b1):
                    for t in range(NT):
                        kk = int(k[t, b, half])
                        if kk == 0:
                            continue
                        v, dd = lists[(c, t, b, half)]
                        vals[p:p + len(v)] = v
                        dsts[p:p + len(v)] = dd
                        p += kk * BLK
                assert p == Kh * BLK
                # gather index i lives at partition i%16, col i//16 (x8 copies)
                gidx[c, :, g0:g0 + Kh * 8] = np.tile(
                    vals.reshape(-1, 16).T, (8, 1))
                dst_t[c, :, col0 + hoff:col0 + hoff + Kh] = (
                    dsts.reshape(Kh, BLK).T.astype(bf16))

    sched = dict(k=k, cols=cols, sb_bounds=sb_bounds, sb_info=sb_info,
                 K_TOT=K_TOT, nblk=nblk, shard=shard)
    return gidx, dst_t, sched


def fuse_weights(p):
    f = np.float32
    W_uf = (p["umW"] @ p["uW1"]).astype(f)
    oW = p["oW"]
    Wb2o = (p["bW2"] @ oW[0:128]).astype(f)
    Wr2o = (p["rW2"] @ oW[128:256]).astype(f)
    Wu2o = (p["uW2"] @ oW[256:384]).astype(f)
    bu_f = (p["ub1"] + p["umb"] @ p["uW1"]).astype(f)
    ob_f = (p["ob"] + p["bb2"] @ oW[0:128] + p["rb2"] @ oW[128:256]
            + p["ub2"] @ oW[256:384]).astype(f)
    weights = np.concatenate(
        [p["bW1"], p["rW1"], p["uW1"], W_uf[:128], W_uf[128:],
         Wb2o, Wr2o, Wu2o], axis=1).astype(f)
    biases = np.stack([p["bb1"], p["rb1"], bu_f, ob_f], axis=1).astype(f)
    return weights, biases


# ---------------------------------------------------------------- kernel build
def build(sched, n):
    k, cols = sched["k"], sched["cols"]
    sb_bounds, sb_info = sched["sb_bounds"], sched["sb_info"]
    K_TOT, nblk, shard = sched["K_TOT"], sched["nblk"], sched["shard"]
    shard_pad = nblk * BLK

    nc = bacc.Bacc(None, target_bir_lowering=False, debug=False)
    x16 = nc.dram_tensor("x16", [n, D], BF16, kind="ExternalInput")
    xsb = nc.dram_tensor("xsb", [BLK, nblk, D], BF16, kind="ExternalInput")
    gidx = nc.dram_tensor("gidx", [BLK, K_TOT * 8], I16, kind="ExternalInput")
    dstv = nc.dram_tensor("dstv", [BLK, K_TOT], BF16, kind="ExternalInput")
    wts = nc.dram_tensor("wts", [BLK, 8 * BLK], F32, kind="ExternalInput")
    bia = nc.dram_tensor("bia", [BLK, 4], F32, kind="ExternalInput")
    iota = nc.dram_tensor("iota", [BLK, BLK], BF16, kind="ExternalInput")
    id16 = nc.dram_tensor("id16", [BLK, BLK], BF16, kind="ExternalInput")
    id32 = nc.dram_tensor("id32", [BLK, BLK], F32, kind="ExternalInput")
    outp = nc.dram_tensor("out", [shard_pad, D], F32, kind="ExternalOutput")

    relu = mybir.ActivationFunctionType.Relu
    with TileContext(nc) as tc:
        with (
            tc.tile_pool(name="const", bufs=1) as cp,
            tc.tile_pool(name="gather", bufs=2) as gp,
            tc.tile_pool(name="slabs", bufs=2) as sp,
            tc.tile_pool(name="outs", bufs=4) as op,
            tc.tile_pool(name="psA", bufs=1, space="PSUM") as psA,
            tc.tile_pool(name="psB", bufs=1, space="PSUM") as psB,
        ):
            gidx_s = cp.tile([BLK, K_TOT * 8], I16)
            nc.sync.dma_start(out=gidx_s[:], in_=gidx[:, :])
            dst_s = cp.tile([BLK, K_TOT], BF16)
            nc.sync.dma_start(out=dst_s[:], in_=dstv[:, :])
            wts_s = cp.tile([BLK, 8 * BLK], F32)
            nc.sync.dma_start(out=wts_s[:], in_=wts[:, :])
            bia_s = cp.tile([BLK, 4], F32)
            nc.sync.dma_start(out=bia_s[:], in_=bia[:, :])
            iota_s = cp.tile([BLK, BLK], BF16)
            nc.sync.dma_start(out=iota_s[:], in_=iota[:, :])
            id16_s = cp.tile([BLK, BLK], BF16)
            nc.sync.dma_start(out=id16_s[:], in_=id16[:, :])
            id32_s = cp.tile([BLK, BLK], F32)
            nc.sync.dma_start(out=id32_s[:], in_=id32[:, :])

            w = {nm: wts_s[:, i * BLK:(i + 1) * BLK]
                 for i, nm in enumerate(["bW1", "rW1", "uW1", "W_uf_a", "W_uf_b",
                                         "Wb2o", "Wr2o", "Wu2o"])}

            for s in range(len(sb_bounds) - 1):
                b0, b1 = sb_bounds[s], sb_bounds[s + 1]
                nb = b1 - b0
                nsb = nb * BLK
                col0, Klo, Khi, glo, ghi = sb_info[s]
                Ks = Klo + Khi

                slab = gp.tile([BLK, Ks, D], BF16, tag="slab")
                if Klo:
                    nc.gpsimd.dma_gather(
                        slab[:, 0:Klo, :], x16[0:min(SPLIT, n), :],
                        gidx_s[:, glo:glo + Klo * 8],
                        Klo * BLK, Klo * BLK, D, single_packet=False)
                if Khi:
                    nc.gpsimd.dma_gather(
                        slab[:, Klo:Ks, :], x16[SPLIT:n, :],
                        gidx_s[:, ghi:ghi + Khi * 8],
                        Khi * BLK, Khi * BLK, D, single_packet=False)
                xsl = gp.tile([BLK, nb, D], BF16, tag="xsl")
                nc.sync.dma_start(out=xsl[:], in_=xsb[:, b0:b1, :])
                A = gp.tile([BLK, Ks, D], BF16, tag="A")
                nc.vector.tensor_tensor(
                    out=A[:],
                    in0=iota_s[:, None, :].to_broadcast([BLK, Ks, D]),
                    in1=dst_s[:, col0:col0 + Ks, None].to_broadcast([BLK, Ks, D]),
                    op=mybir.AluOpType.is_equal,
                )

                ps_x = psA.tile([D, nsb], F32, tag="ps_x")
                ps = {t: psA.tile([D, nsb], F32, tag=f"ps_{t}", name=f"ps_{t}_{s}")
                      for t in range(NT)}

                for b in range(b0, b1):
                    bc = b - b0
                    sl = bass.ts(bc, BLK)
                    nc.tensor.matmul(out=ps_x[:, sl], lhsT=xsl[:, bc, :],
                                     rhs=id16_s[:, :], start=True, stop=True)
                    for t in range(NT):
                        cl = cols[(b, t)]
                        for j, cg in enumerate(cl):
                            lc = cg - col0
                            nc.tensor.matmul(out=ps[t][:, sl],
                                             lhsT=slab[:, lc, :],
                                             rhs=A[:, lc, :],
                                             start=(j == 0),
                                             stop=(j == len(cl) - 1))

                xT_s = sp.tile([D, nsb], F32, tag="xT")
                nc.vector.tensor_copy(out=xT_s[:], in_=ps_x[:])
                b_s = sp.tile([D, nsb], F32, tag="b_s")
                nc.scalar.copy(out=b_s[:], in_=ps[0][:])
                r_s = sp.tile([D, nsb], F32, tag="r_s")
                nc.scalar.copy(out=r_s[:], in_=ps[1][:])
                u1_s = sp.tile([D, nsb], F32, tag="u1_s")
                nc.vector.tensor_copy(out=u1_s[:], in_=ps[2][:])
                u2_s = sp.tile([D, nsb], F32, tag="u2_s")
                nc.vector.tensor_copy(out=u2_s[:], in_=ps[3][:])

                h1b_p = psB.tile([D, nsb], F32, tag="h1")
                nc.tensor.matmul(out=h1b_p[:], lhsT=w["bW1"], rhs=b_s[:],
                                 start=True, stop=True)
                h1b_s = sp.tile([D, nsb], F32, tag="h1b")
                nc.scalar.activation(out=h1b_s[:], in_=h1b_p[:], func=relu,
                                     bias=bia_s[:, 0:1])
                h1r_p = psB.tile([D, nsb], F32, tag="h1")
                nc.tensor.matmul(out=h1r_p[:], lhsT=w["rW1"], rhs=r_s[:],
                                 start=True, stop=True)
                h1r_s = sp.tile([D, nsb], F32, tag="h1r")
                nc.scalar.activation(out=h1r_s[:], in_=h1r_p[:], func=relu,
                                     bias=bia_s[:, 1:2])
                h1u_p = psB.tile([D, nsb], F32, tag="h1")
                nc.tensor.matmul(out=h1u_p[:], lhsT=w["uW1"], rhs=xT_s[:],
                                 start=True, stop=False)
                nc.tensor.matmul(out=h1u_p[:], lhsT=w["W_uf_a"], rhs=u1_s[:],
                                 start=False, stop=False)
                nc.tensor.matmul(out=h1u_p[:], lhsT=w["W_uf_b"], rhs=u2_s[:],
                                 start=False, stop=True)
                h1u_s = sp.tile([D, nsb], F32, tag="h1u")
                nc.scalar.activation(out=h1u_s[:], in_=h1u_p[:], func=relu,
                                     bias=bia_s[:, 2:3])

                out_p = psB.tile([D, nsb], F32, tag="outp")
                nc.tensor.matmul(out=out_p[:], lhsT=w["Wb2o"], rhs=h1b_s[:],
                                 start=True, stop=False)
                nc.tensor.matmul(out=out_p[:], lhsT=w["Wr2o"], rhs=h1r_s[:],
                                 start=False, stop=False)
                nc.tensor.matmul(out=out_p[:], lhsT=w["Wu2o"], rhs=h1u_s[:],
                                 start=False, stop=True)
                outT_s = sp.tile([D, nsb], F32, tag="outT")
                nc.scalar.activation(out=outT_s[:], in_=out_p[:], func=relu,
                                     bias=bia_s[:, 3:4])

                for bc in range(nb):
                    tr_p = psB.tile([BLK, BLK], F32, tag="tr")
                    nc.tensor.transpose(out=tr_p[:],
                                        in_=outT_s[:, bass.ts(bc, BLK)],
                                        identity=id32_s[:, :])
                    onat = op.tile([BLK, BLK], F32, tag="onat")
                    if bc % 2 == 0:
                        nc.vector.tensor_copy(out=onat[:], in_=tr_p[:])
                    else:
                        nc.scalar.copy(out=onat[:], in_=tr_p[:])
                    nc.sync.dma_start(out=outp[(b0 + bc) * BLK:(b0 + bc + 1) * BLK, :],
                                      in_=onat[:])
    nc.compile()
    return nc


# ---------------------------------------------------------------- entry point
def kernel(x, edge_index, edge_type, upper_ind, cell_dimension,
           bW1, bb1, bW2, bb2, rW1, rb1, rW2, rb2,
           umW, umb, uW1, ub1, uW2, ub2, oW, ob, _trace=False):
    global LAST_EXEC_NS, LAST_TRACE_PATH
    params = dict(bW1=bW1, bb1=bb1, bW2=bW2, bb2=bb2, rW1=rW1, rb1=rb1,
                  rW2=rW2, rb2=rb2, umW=umW, umb=umb, uW1=uW1, ub1=ub1,
                  uW2=uW2, ub2=ub2, oW=oW, ob=ob)
    params = {k_: np.asarray(v, np.float32) for k_, v in params.items()}
    x = np.asarray(x, np.float32)
    src = np.asarray(edge_index[0], np.int64)
    dst = np.asarray(edge_index[1], np.int64)
    et = np.asarray(edge_type, np.int64)
    ui = np.asarray(upper_ind, np.int64)

    shard, nblk, shard_pad = _cfg(N, NC)
    gidx_t, dst_t, sched = preprocess(src, dst, et, ui, N, NC)
    weights, biases = fuse_weights(params)

    x16 = x.astype(bf16)
    xsb_all = []
    for c in range(NC):
        rows = c * shard + np.minimum(np.arange(shard_pad), shard - 1)
        xsb_all.append(np.ascontiguousarray(
            x16[rows].reshape(nblk, BLK, D).transpose(1, 0, 2)))

    iota_np = np.broadcast_to(np.arange(BLK, dtype=np.float32), (BLK, BLK)
                              ).astype(bf16)
    ident = np.eye(BLK, dtype=np.float32)

    nc = build(sched, N)

    in_maps = []
    for c in range(NC):
        in_maps.append({
            "x16": x16, "xsb": xsb_all[c], "gidx": gidx_t[c], "dstv": dst_t[c],
            "wts": weights, "bia": biases, "iota": np.ascontiguousarray(iota_np),
            "id16": ident.astype(bf16), "id32": ident,
        })
    res = run_bass_kernel_spmd(nc, in_maps, core_ids=list(range(NC)),
                               trace=_trace, trace_cores=list(range(NC)))
    LAST_EXEC_NS = res.exec_time_ns
    if res.instructions_and_trace is not None:
        LAST_TRACE_PATH = res.instructions_and_trace[1]
    out = np.concatenate([res.results[c]["out"][:shard] for c in range(NC)], 0)
    return out.astype(np.float32)


if __name__ == "__main__":
    import reference
    inp = {k_: np.asarray(v) for k_, v in reference.setup_inputs().items()}
    got = kernel(**inp)
    exp = np.asarray(reference.reference(**inp))
    print(f"Relative error: {np.linalg.norm(got - exp) / np.linalg.norm(exp):.4e}")


# revision 8
# speedup vs baseline: 1.7797x; 1.7797x over previous
"""Trainium2 Bass kernel for CINConv-style GNN message passing.

Strategy (8 NeuronCores, data parallel over destination nodes):
  - Core c owns nodes [c*6250, (c+1)*6250). Edges are partitioned by their
    destination shard on the host, then bucketed by (type, 128-node block).
  - Source rows are gathered from a bf16 copy of x with `dma_gather`
    (Q7 SWDGE gather, int16 indices -> x is addressed as two halves), 128
    edges per chunk, and aggregated into PSUM with one-hot matmuls:
        agg^T[d, n] += feat[e, d]^T @ onehot[e, n]
    so aggregates land directly in transposed [feat, node] layout.
  - Self terms (the (1+eps)*x contributions) ride along as self-edges.
  - The per-node MLPs run in transposed layout; second-layer weights are
    fused into the output projection on the host:
        out = relu(h1b @ (bW2 oW_b) + h1r @ (rW2 oW_r) + h1u @ (uW2 oW_u) + ob')
  - Results are PE-transposed back to [node, feat] and stored.
"""

import numpy as np
import ml_dtypes

import concourse.bass as bass
import concourse.mybir as mybir
from concourse import bacc
from concourse.tile import TileContext
from concourse.bass_utils import run_bass_kernel_spmd

bf16 = ml_dtypes.bfloat16
F32 = mybir.dt.float32
BF16 = mybir.dt.bfloat16
I16 = mybir.dt.int16

# ---- problem config (hardcoded) ----
N, E, D = 50000, 800000, 128
NC = 8
BLK = 128
PAD_DST = 200.0
SPLIT = 32768  # int16 index limit: x rows addressed as [0,SPLIT) + [SPLIT,N)
BOUNDARY, UPPER, REWIRE = 0, 1, 2
NT = 4  # chunk types: B, R, U1 (src half of upper msg), U2 (upper_ind half)

LAST_EXEC_NS = None
LAST_TRACE_PATH = None


def _cfg(n, n_cores):
    shard = n // n_cores
    nblk = -(-shard // BLK)
    return shard, nblk, nblk * BLK


# ---------------------------------------------------------------- host prep
def preprocess(src, dst, et, ui, n, n_cores, sb_blocks=4):
    """Bucket edges by (core, type, block); split by source half; build the
    shared chunk schedule plus per-core gather-index / one-hot-dst tensors."""
    shard, nblk, _ = _cfg(n, n_cores)
    core_of = dst // shard
    dloc = dst - core_of * shard
    blk = dloc // BLK
    doff = dloc - blk * BLK

    tmap = np.full(3, -1, np.int64)
    tmap[BOUNDARY], tmap[REWIRE], tmap[UPPER] = 0, 1, 2
    t_of = tmap[et]

    key = (core_of * 3 + t_of) * nblk + blk
    order = np.argsort(key, kind="stable")
    key_s = key[order]
    src_s, doff_s, ui_s = src[order], doff[order], ui[order]
    starts = np.searchsorted(key_s, np.arange(n_cores * 3 * nblk + 1))

    def bucket(c, t, b):
        i0, i1 = starts[(c * 3 + t) * nblk + b], starts[(c * 3 + t) * nblk + b + 1]
        return i0, i1

    # per (core, chunk-type, block, half) edge (val, dst) lists
    # chunk types: 0=B (self+boundary), 1=R (self+rewire), 2=U1 (src),
    #              3=U2 (upper_ind); halves: 0=lo (<SPLIT), 1=hi
    lists = {}
    cnt = np.zeros((n_cores, NT, nblk, 2), np.int64)
    for c in range(n_cores):
        for b in range(nblk):
            i0, i1 = bucket(c, 0, b)
            vB, dB = src_s[i0:i1], doff_s[i0:i1]
            i0, i1 = bucket(c, 1, b)
            vR, dR = src_s[i0:i1], doff_s[i0:i1]
            i0, i1 = bucket(c, 2, b)
            vU1, dU1 = src_s[i0:i1], doff_s[i0:i1]
            vU2, dU2 = ui_s[i0:i1], doff_s[i0:i1]
            for t, (v, dd) in enumerate([(vB, dB), (vR, dR), (vU1, dU1), (vU2, dU2)]):
                m = v < SPLIT
                lists[(c, t, b, 0)] = (v[m], dd[m])
                lists[(c, t, b, 1)] = (v[~m] - SPLIT, dd[~m])
                cnt[c, t, b, 0] = m.sum()
                cnt[c, t, b, 1] = (~m).sum()

    # shared schedule: chunks per (type, block, half); U1/U2 need >=1 chunk
    k = -(-cnt.max(axis=0) // BLK)          # [NT, nblk, 2]
    empty = k.sum(axis=2) == 0
    empty[0:2, :] = False                   # B/R init via identity matmul
    k[:, :, 0][empty] = 1

    # column layout: per super-block: [lo cols (b-major, t-minor)][hi cols]
    sb_bounds = list(range(0, nblk, sb_blocks)) + [nblk]
    nsb_count = len(sb_bounds) - 1
    cols = {}            # (b, t) -> list of global slab/dst cols (lo then hi)
    sb_info = []         # per sb: (col0, Klo, Khi, gidx_lo0, gidx_hi0)
    col = 0
    gcol = 0
    for s in range(nsb_count):
        b0, b1 = sb_bounds[s], sb_bounds[s + 1]
        col0 = col
        for b in range(b0, b1):
            for t in range(NT):
                cols[(b, t)] = [col + j for j in range(int(k[t, b, 0]))]
                col += int(k[t, b, 0])
        Klo = col - col0
        for b in range(b0, b1):
            for t in range(NT):
                cols[(b, t)] = cols[(b, t)] + [col + j
                                               for j in range(int(k[t, b, 1]))]
                col += int(k[t, b, 1])
        Khi = col - col0 - Klo
        sb_info.append((col0, Klo, Khi, gcol, gcol + Klo * 8))
        gcol += (Klo + Khi) * 8
    K_TOT = col

    # gather pieces: (sb, half, col_lo, col_hi) split for SWDGE queue overlap
    pieces = []
    for s in range(nsb_count):
        col0, Klo, Khi, glo, ghi = sb_info[s]
        for half, Kh in [(0, Klo), (1, Khi)]:
            if Kh == 0:
                continue
            nsplit = 2 if Kh >= 8 else 1
            cut = -(-Kh // nsplit)
            for c0 in range(0, Kh, cut):
                pieces.append((s, half, c0, min(c0 + cut, Kh)))

    gidx = np.zeros((n_cores, BLK, K_TOT * 8), np.int16)
    dst_t = np.full((n_cores, BLK, K_TOT), PAD_DST, bf16)

    for c in range(n_cores):
        for s in range(nsb_count):
            b0, b1 = sb_bounds[s], sb_bounds[s + 1]
            col0, Klo, Khi, glo, ghi = sb_info[s]
            for half, Kh, g0, hoff in [(0, Klo, glo, 0), (1, Khi, ghi, Klo)]:
                if Kh == 0:
                    continue
                vals = np.zeros(Kh * BLK, np.int16)
                dsts = np.full(Kh * BLK, PAD_DST, np.float32)
                p = 0
                for b in range(b0, b1):
                    for t in range(NT):
                        kk = int(k[t, b, half])
                        if kk == 0:
                            continue
                        v, dd = lists[(c, t, b, half)]
                        vals[p:p + len(v)] = v
                        dsts[p:p + len(v)] = dd
                        p += kk * BLK
                assert p == Kh * BLK
                # gather index i lives at partition i%16, col i//16 (x8 copies)
                gidx[c, :, g0:g0 + Kh * 8] = np.tile(
                    vals.reshape(-1, 16).T, (8, 1))
                dst_t[c, :, col0 + hoff:col0 + hoff + Kh] = (
                    dsts.reshape(Kh, BLK).T.astype(bf16))

    sched = dict(k=k, cols=cols, sb_bounds=sb_bounds, sb_info=sb_info,
                 K_TOT=K_TOT, nblk=nblk, shard=shard, pieces=pieces)
    return gidx, dst_t, sched


def fuse_weights(p):
    f = np.float32
    W_uf = (p["umW"] @ p["uW1"]).astype(f)
    oW = p["oW"]
    Wb2o = (p["bW2"] @ oW[0:128]).astype(f)
    Wr2o = (p["rW2"] @ oW[128:256]).astype(f)
    Wu2o = (p["uW2"] @ oW[256:384]).astype(f)
    bu_f = (p["ub1"] + p["umb"] @ p["uW1"]).astype(f)
    ob_f = (p["ob"] + p["bb2"] @ oW[0:128] + p["rb2"] @ oW[128:256]
            + p["ub2"] @ oW[256:384]).astype(f)
    weights = np.concatenate(
        [p["bW1"], p["rW1"], p["uW1"], W_uf[:128], W_uf[128:],
         Wb2o, Wr2o, Wu2o], axis=1).astype(f)
    biases = np.stack([p["bb1"], p["rb1"], bu_f, ob_f], axis=1).astype(f)
    return weights, biases


# ---------------------------------------------------------------- kernel build
def build(sched, n):
    k, cols = sched["k"], sched["cols"]
    pieces = sched["pieces"]
    qrr = [0]
    sb_bounds, sb_info = sched["sb_bounds"], sched["sb_info"]
    K_TOT, nblk, shard = sched["K_TOT"], sched["nblk"], sched["shard"]
    shard_pad = nblk * BLK

    nc = bacc.Bacc(None, target_bir_lowering=False, debug=False,
                   num_swdge_queues=4)
    x16 = nc.dram_tensor("x16", [n, D], BF16, kind="ExternalInput")
    xsb = nc.dram_tensor("xsb", [BLK, nblk, D], BF16, kind="ExternalInput")
    gidx = nc.dram_tensor("gidx", [BLK, K_TOT * 8], I16, kind="ExternalInput")
    dstv = nc.dram_tensor("dstv", [BLK, K_TOT], BF16, kind="ExternalInput")
    wts = nc.dram_tensor("wts", [BLK, 8 * BLK], F32, kind="ExternalInput")
    bia = nc.dram_tensor("bia", [BLK, 4], F32, kind="ExternalInput")
    iota = nc.dram_tensor("iota", [BLK, BLK], BF16, kind="ExternalInput")
    id16 = nc.dram_tensor("id16", [BLK, BLK], BF16, kind="ExternalInput")
    id32 = nc.dram_tensor("id32", [BLK, BLK], F32, kind="ExternalInput")
    outp = nc.dram_tensor("out", [shard_pad, D], F32, kind="ExternalOutput")

    relu = mybir.ActivationFunctionType.Relu
    with TileContext(nc) as tc:
        with (
            tc.tile_pool(name="const", bufs=1) as cp,
            tc.tile_pool(name="gather", bufs=2) as gp,
            tc.tile_pool(name="slabs", bufs=2) as sp,
            tc.tile_pool(name="outs", bufs=4) as op,
            tc.tile_pool(name="psA", bufs=1, space="PSUM") as psA,
            tc.tile_pool(name="psB", bufs=1, space="PSUM") as psB,
        ):
            gidx_s = cp.tile([BLK, K_TOT * 8], I16)
            nc.sync.dma_start(out=gidx_s[:], in_=gidx[:, :])
            dst_s = cp.tile([BLK, K_TOT], BF16)
            nc.sync.dma_start(out=dst_s[:], in_=dstv[:, :])
            wts_s = cp.tile([BLK, 8 * BLK], F32)
            nc.sync.dma_start(out=wts_s[:], in_=wts[:, :])
            bia_s = cp.tile([BLK, 4], F32)
            nc.sync.dma_start(out=bia_s[:], in_=bia[:, :])
            iota_s = cp.tile([BLK, BLK], BF16)
            nc.sync.dma_start(out=iota_s[:], in_=iota[:, :])
            id16_s = cp.tile([BLK, BLK], BF16)
            nc.sync.dma_start(out=id16_s[:], in_=id16[:, :])
            id32_s = cp.tile([BLK, BLK], F32)
            nc.sync.dma_start(out=id32_s[:], in_=id32[:, :])

            w = {nm: wts_s[:, i * BLK:(i + 1) * BLK]
                 for i, nm in enumerate(["bW1", "rW1", "uW1", "W_uf_a", "W_uf_b",
                                         "Wb2o", "Wr2o", "Wu2o"])}

            for s in range(len(sb_bounds) - 1):
                b0, b1 = sb_bounds[s], sb_bounds[s + 1]
                nb = b1 - b0
                nsb = nb * BLK
                col0, Klo, Khi, glo, ghi = sb_info[s]
                Ks = Klo + Khi

                slab = gp.tile([BLK, Ks, D], BF16, tag="slab")
                for (ps_, half, c0, c1) in pieces:
                    if ps_ != s:
                        continue
                    npc = (c1 - c0) * BLK
                    if half == 0:
                        nc.gpsimd.dma_gather(
                            slab[:, c0:c1, :], x16[0:min(SPLIT, n), :],
                            gidx_s[:, glo + c0 * 8:glo + c1 * 8],
                            npc, npc, D, single_packet=False,
                            queue_num=qrr[0] % 4)
                    else:
                        nc.gpsimd.dma_gather(
                            slab[:, Klo + c0:Klo + c1, :], x16[SPLIT:n, :],
                            gidx_s[:, ghi + c0 * 8:ghi + c1 * 8],
                            npc, npc, D, single_packet=False,
                            queue_num=qrr[0] % 4)
                    qrr[0] += 1
                xsl = gp.tile([BLK, nb, D], BF16, tag="xsl")
                nc.sync.dma_start(out=xsl[:], in_=xsb[:, b0:b1, :])
                A = gp.tile([BLK, Ks, D], BF16, tag="A")
                nc.vector.tensor_tensor(
                    out=A[:],
                    in0=iota_s[:, None, :].to_broadcast([BLK, Ks, D]),
                    in1=dst_s[:, col0:col0 + Ks, None].to_broadcast([BLK, Ks, D]),
                    op=mybir.AluOpType.is_equal,
                )

                ps_x = psA.tile([D, nsb], F32, tag="ps_x")
                ps = {t: psA.tile([D, nsb], F32, tag=f"ps_{t}", name=f"ps_{t}_{s}")
                      for t in range(NT)}

                for b in range(b0, b1):
                    bc = b - b0
                    sl = bass.ts(bc, BLK)
                    nc.tensor.matmul(out=ps_x[:, sl], lhsT=xsl[:, bc, :],
                                     rhs=id16_s[:, :], start=True, stop=True)
                    for t in range(NT):
                        cl = cols[(b, t)]
                        if t < 2:
                            # x term of (1+eps)*x + agg rides as an identity MM
                            nc.tensor.matmul(out=ps[t][:, sl],
                                             lhsT=xsl[:, bc, :],
                                             rhs=id16_s[:, :],
                                             start=True, stop=(not cl))
                        for j, cg in enumerate(cl):
                            lc = cg - col0
                            nc.tensor.matmul(out=ps[t][:, sl],
                                             lhsT=slab[:, lc, :],
                                             rhs=A[:, lc, :],
                                             start=(t >= 2 and j == 0),
                                             stop=(j == len(cl) - 1))

                xT_s = sp.tile([D, nsb], F32, tag="xT")
                nc.vector.tensor_copy(out=xT_s[:], in_=ps_x[:])
                b_s = sp.tile([D, nsb], F32, tag="b_s")
                nc.scalar.copy(out=b_s[:], in_=ps[0][:])
                r_s = sp.tile([D, nsb], F32, tag="r_s")
                nc.scalar.copy(out=r_s[:], in_=ps[1][:])
                u1_s = sp.tile([D, nsb], F32, tag="u1_s")
                nc.vector.tensor_copy(out=u1_s[:], in_=ps[2][:])
                u2_s = sp.tile([D, nsb], F32, tag="u2_s")
                nc.vector.tensor_copy(out=u2_s[:], in_=ps[3][:])

                h1b_p = psB.tile([D, nsb], F32, tag="h1")
                nc.tensor.matmul(out=h1b_p[:], lhsT=w["bW1"], rhs=b_s[:],
                                 start=True, stop=True)
                h1b_s = sp.tile([D, nsb], F32, tag="h1b")
                nc.scalar.activation(out=h1b_s[:], in_=h1b_p[:], func=relu,
                                     bias=bia_s[:, 0:1])
                h1r_p = psB.tile([D, nsb], F32, tag="h1")
                nc.tensor.matmul(out=h1r_p[:], lhsT=w["rW1"], rhs=r_s[:],
                                 start=True, stop=True)
                h1r_s = sp.tile([D, nsb], F32, tag="h1r")
                nc.scalar.activation(out=h1r_s[:], in_=h1r_p[:], func=relu,
                                     bias=bia_s[:, 1:2])
                h1u_p = psB.tile([D, nsb], F32, tag="h1")
                nc.tensor.matmul(out=h1u_p[:], lhsT=w["uW1"], rhs=xT_s[:],
                                 start=True, stop=False)
                nc.tensor.matmul(out=h1u_p[:], lhsT=w["W_uf_a"], rhs=u1_s[:],
                                 start=False, stop=False)
                nc.tensor.matmul(out=h1u_p[:], lhsT=w["W_uf_b"], rhs=u2_s[:],
                                 start=False, stop=True)
                h1u_s = sp.tile([D, nsb], F32, tag="h1u")
                nc.scalar.activation(out=h1u_s[:], in_=h1u_p[:], func=relu,
                                     bias=bia_s[:, 2:3])

                out_p = psB.tile([D, nsb], F32, tag="outp")
                nc.tensor.matmul(out=out_p[:], lhsT=w["Wb2o"], rhs=h1b_s[:],
                                 start=True, stop=False)
                nc.tensor.matmul(out=out_p[:], lhsT=w["Wr2o"], rhs=h1r_s[:],
                                 start=False, stop=False)
                nc.tensor.matmul(out=out_p[:], lhsT=w["Wu2o"], rhs=h1u_s[:],
                                 start=False, stop=True)
                outT_s = sp.tile([D, nsb], F32, tag="outT")
                nc.scalar.activation(out=outT_s[:], in_=out_p[:], func=relu,
                                     bias=bia_s[:, 3:4])

                for bc in range(nb):
                    tr_p = psB.tile([BLK, BLK], F32, tag="tr")
                    nc.tensor.transpose(out=tr_p[:],
                                        in_=outT_s[:, bass.ts(bc, BLK)],
                                        identity=id32_s[:, :])
                    onat = op.tile([BLK, BLK], F32, tag="onat")
                    if bc % 2 == 0:
                        nc.vector.tensor_copy(out=onat[:], in_=tr_p[:])
                    else:
                        nc.scalar.copy(out=onat[:], in_=tr_p[:])
                    nc.sync.dma_start(out=outp[(b0 + bc) * BLK:(b0 + bc + 1) * BLK, :],
                                      in_=onat[:])
    nc.compile()
    return nc


# ---------------------------------------------------------------- entry point
def kernel(x, edge_index, edge_type, upper_ind, cell_dimension,
           bW1, bb1, bW2, bb2, rW1, rb1, rW2, rb2,
           umW, umb, uW1, ub1, uW2, ub2, oW, ob, _trace=False):
    global LAST_EXEC_NS, LAST_TRACE_PATH
    params = dict(bW1=bW1, bb1=bb1, bW2=bW2, bb2=bb2, rW1=rW1, rb1=rb1,
                  rW2=rW2, rb2=rb2, umW=umW, umb=umb, uW1=uW1, ub1=ub1,
                  uW2=uW2, ub2=ub2, oW=oW, ob=ob)
    params = {k_: np.asarray(v, np.float32) for k_, v in params.items()}
    x = np.asarray(x, np.float32)
    src = np.asarray(edge_index[0], np.int64)
    dst = np.asarray(edge_index[1], np.int64)
    et = np.asarray(edge_type, np.int64)
    ui = np.asarray(upper_ind, np.int64)

    shard, nblk, shard_pad = _cfg(N, NC)
    gidx_t, dst_t, sched = preprocess(src, dst, et, ui, N, NC)
    weights, biases = fuse_weights(params)

    x16 = x.astype(bf16)
    xsb_all = []
    for c in range(NC):
        rows = c * shard + np.minimum(np.arange(shard_pad), shard - 1)
        xsb_all.append(np.ascontiguousarray(
            x16[rows].reshape(nblk, BLK, D).transpose(1, 0, 2)))

    iota_np = np.broadcast_to(np.arange(BLK, dtype=np.float32), (BLK, BLK)
                              ).astype(bf16)
    ident = np.eye(BLK, dtype=np.float32)

    nc = build(sched, N)

    in_maps = []
    for c in range(NC):
        in_maps.append({
            "x16": x16, "xsb": xsb_all[c], "gidx": gidx_t[c], "dstv": dst_t[c],
            "wts": weights, "bia": biases, "iota": np.ascontiguousarray(iota_np),
            "id16": ident.astype(bf16), "id32": ident,
        })
    res = run_bass_kernel_spmd(nc, in_maps, core_ids=list(range(NC)),
                               trace=_trace, trace_cores=list(range(NC)))
    LAST_EXEC_NS = res.exec_time_ns
    if res.instructions_and_trace is not None:
        LAST_TRACE_PATH = res.instructions_and_trace[1]
    out = np.concatenate([res.results[c]["out"][:shard] for c in range(NC)], 0)
    return out.astype(np.float32)


if __name__ == "__main__":
    import reference
    inp = {k_: np.asarray(v) for k_, v in reference.setup_inputs().items()}
    got = kernel(**inp)
    exp = np.asarray(reference.reference(**inp))
    print(f"Relative error: {np.linalg.norm(got - exp) / np.linalg.norm(exp):.4e}")
